# revision 42
# baseline (speedup 1.0000x reference)
"""TRN2 Bass kernel for nn_CustomHeadMultiHeadAttention (dense transformer).

Full inputs: x [8, 2048, 1024] f32 + QKV/classify weights. Sharding: pure
data parallelism — batch 8 across 8 NeuronCores, one batch element per core.
Each core runs the complete MHA + GELU + classify on its slice; no
collectives. Host only slices the batch and stacks/transposes the outputs.

Per-core pipeline (bf16 matmul operands, fp32 PSUM accumulation), v2:
  startup: PE-transposes of x seq-tiles, q0/k0 projection blocks and the
    first half of V are interleaved so the PE chases the x DMA stream
    instead of FIFO-stalling on the last tile.
  per head h, per 512-wide q block (software-pipelined one block deep):
    scores^T tiles = kh-chunk (lhsT) @ qh      [k=128, q=512] x2 per group
    P^T  = exp(scores^T / sqrt(dk))            ACT PSUM->SBUF bf16
    PV of the PREVIOUS block is interleaved between score groups, so the
    PE never head-of-line blocks on the exp stream.
    denom = 5-op fused bf16 tree over P^T + twos[128,128] @ t1 broadcast
    (the 2x folds the 0.5 of gelu into the softmax reciprocal)
    an'  = attn^T/2 = pv * reciprocal_approx_fast(2*denom)
  per head epilogue (no ACT table switches: exp/tanh/copy share one set):
    gelu_sigmoid: t = tanh(1.7018*an'); s = an' + an'*t  == gelu(attn)
    classify partial: clacc[2, S] += Wc-chunk(h) lhsT @ s   (DVE accum)
  out = clacc + bc (accumulated from head 0), single DMA [2, S].
Host transposes logits^T [2, s] -> [2048, 2].
"""

import math
import sys
from collections import deque

sys.path.insert(0, "/opt/trn_rl_repo")

import numpy as np

import concourse.bass as bass
import concourse.mybir as mybir
import concourse.tile as tile
from concourse import bacc
from concourse.bass_utils import run_bass_kernel_spmd
from concourse.masks import make_identity

AF = mybir.ActivationFunctionType
ALU = mybir.AluOpType
F32 = mybir.dt.float32
BF16 = mybir.dt.bfloat16

B = 8           # batch (== number of cores)
S = 2048        # sequence length
H = 1024        # hidden
NH = 8          # heads
DK = 128        # head dim
P = 128         # partitions
NC = 2          # classes
SB = S // 512   # 4 q/s blocks of 512
HT = H // P     # 8 hidden tiles
ST = S // P     # 16 seq tiles
SCALE = 1.0 / math.sqrt(DK)
TANH_SCALE = 1.7018  # tanh arg on an' = attn/2: tanh(0.8509 * attn)

_NC_CACHE = []


def _build():
    nc = bacc.Bacc(None, target_bir_lowering=False, debug=False)

    x = nc.dram_tensor("x", [S, H], F32, kind="ExternalInput")
    Wq = nc.dram_tensor("Wq", [H, H], F32, kind="ExternalInput")
    bq = nc.dram_tensor("bq", [H], F32, kind="ExternalInput")
    Wk = nc.dram_tensor("Wk", [H, H], F32, kind="ExternalInput")
    bk = nc.dram_tensor("bk", [H], F32, kind="ExternalInput")
    Wv = nc.dram_tensor("Wv", [H, H], F32, kind="ExternalInput")
    bv = nc.dram_tensor("bv", [H], F32, kind="ExternalInput")
    Wc = nc.dram_tensor("Wc", [H, NC], F32, kind="ExternalInput")
    bc = nc.dram_tensor("bc", [NC], F32, kind="ExternalInput")
    out = nc.dram_tensor("out", [NC, S], F32, kind="ExternalOutput")

    with tile.TileContext(nc) as tc:
        with (
            tc.tile_pool(name="persist", bufs=1) as persist,
            tc.tile_pool(name="g2ps", bufs=2, space="PSUM") as g2ps,
            tc.tile_pool(name="pvps", bufs=2, space="PSUM") as pvpool,
            tc.tile_pool(name="auxps", bufs=2, space="PSUM") as auxps,
            tc.tile_pool(name="wj", bufs=3) as wjpool,
            tc.tile_pool(name="qk", bufs=2) as qkpool,
        ):
            ident = persist.tile([P, P], F32, tag="ident")
            make_identity(nc, ident)
            twos128 = persist.tile([P, P], BF16, tag="twos128")
            nc.vector.memset(twos128, 2.0)

            xT = persist.tile([P, HT, S], BF16, tag="xT")
            wv_sb = persist.tile([P, HT, H], BF16, tag="wv")
            v_sb = persist.tile([P, ST, H], BF16, tag="v")
            an = persist.tile([P, NH, S], BF16, tag="an")
            wg = persist.tile([P, S], BF16, tag="wg")
            clacc = persist.tile([NC, S], F32, tag="clacc")
            wq_r = Wq.rearrange("(o p) d -> p o d", p=P)
            wk_r = Wk.rearrange("(o p) d -> p o d", p=P)
            wv_r = Wv.rearrange("(o p) d -> p o d", p=P)

            bq_sb = persist.tile([P, HT], F32, tag="bq")
            bk_sb = persist.tile([P, HT], F32, tag="bk")
            bv_bc = persist.tile([P, H], BF16, tag="bv")
            bc_sb = persist.tile([NC, 1], F32, tag="bc")
            wc_sb = persist.tile([P, HT, NC], BF16, tag="wc")

            def alloc_qk(h):
                qh = qkpool.tile([P, S], BF16, tag="qh", name=f"qh{h}")
                kh = qkpool.tile([P, S], BF16, tag="kh", name=f"kh{h}")
                return qh, kh

            def qk_block(h, wj, b_sb, oT, ss):
                ps = auxps.tile([P, 512], F32, tag="aux",
                                name=f"qk{h}_{ss}")
                for hi in range(HT):
                    nc.tensor.matmul(
                        ps,
                        wj[:, hi, :],
                        xT[:, hi, ss * 512:(ss + 1) * 512],
                        start=(hi == 0),
                        stop=(hi == HT - 1),
                    )
                nc.vector.tensor_tensor(
                    oT[:, ss * 512:(ss + 1) * 512],
                    ps,
                    b_sb[:, h:h + 1].to_broadcast((P, 512)),
                    ALU.add,
                )

            def produce_qk(h, tiles):
                """Project q,k for head h; yields per (w, ss) 8-MM block."""
                qh, kh = tiles
                for w_r, b_sb, oT in ((wq_r, bq_sb, qh), (wk_r, bk_sb, kh)):
                    wj = wjpool.tile([P, HT, P], BF16, tag="wj",
                                     name=f"wj{h}")
                    nc.gpsimd.dma_start(wj, w_r[:, :, h * P:(h + 1) * P])
                    for ss in range(4):
                        qk_block(h, wj, b_sb, oT, ss)
                        yield

            def produce_v_half(dh, sts):
                for st in sts:
                    ps = auxps.tile([P, 512], F32, tag="aux",
                                    name=f"v{dh}_{st}")
                    for hi in range(HT):
                        nc.tensor.matmul(
                            ps,
                            xT[:, hi, st * P:(st + 1) * P],
                            wv_sb[:, hi, dh * 512:(dh + 1) * 512],
                            start=(hi == 0),
                            stop=(hi == HT - 1),
                        )
                    nc.vector.tensor_tensor(
                        v_sb[:, st, dh * 512:(dh + 1) * 512],
                        ps,
                        bv_bc[:, dh * 512:(dh + 1) * 512],
                        ALU.add,
                    )
                    yield

            with tc.tile_pool(name="xload", bufs=1) as xload:
                # gpsimd (casting SWDGE) queue order: q0/k0 weights first so
                # the head-0 projection can start early; V's first half next.
                qk_next = alloc_qk(0)
                wjq0 = wjpool.tile([P, HT, P], BF16, tag="wj", name="wj0q")
                nc.gpsimd.dma_start(wjq0, wq_r[:, :, 0:P])
                wjk0 = wjpool.tile([P, HT, P], BF16, tag="wj", name="wj0k")
                nc.gpsimd.dma_start(wjk0, wk_r[:, :, 0:P])
                # flat bias rows on the gpsimd ring (contiguous, cheap even
                # on SWDGE; any extra issue on the sync ring delays the x
                # tile stream the PE chases); PE-transposed to [128,8] later
                bq_f = xload.tile([HT, P], F32, tag="bqf")
                bk_f = xload.tile([HT, P], F32, tag="bkf")
                nc.gpsimd.dma_start(bq_f, bq.rearrange("(j p) -> j p", p=P))
                nc.gpsimd.dma_start(bk_f, bk.rearrange("(j p) -> j p", p=P))
                nc.gpsimd.dma_start(bc_sb, bc[:, None])
                for hi in range(HT):
                    nc.gpsimd.dma_start(wv_sb[:, hi, 0:512],
                                        wv_r[:, hi, 0:512])
                nc.gpsimd.dma_start(bv_bc, bv[None, :].to_broadcast((P, H)))
                for hi in range(HT):
                    nc.gpsimd.dma_start(wv_sb[:, hi, 512:1024],
                                        wv_r[:, hi, 512:1024])
                nc.gpsimd.dma_start(wc_sb, Wc.rearrange("(j p) c -> p j c", p=P))

                # sync (fast HWDGE) queue: nothing but the x stream — the
                # first tiles gate the PE
                xts = []
                for st in range(ST):
                    xt = xload.tile([P, H], F32, tag=f"xl{st % 8}",
                                    name=f"xt{st}")
                    nc.sync.dma_start(xt, x[st * P:(st + 1) * P, :])
                    xts.append(xt)

                def qk0_block256(w, c2):
                    # head-0 q/k in 256-wide blocks: block c2 needs only x
                    # seq tiles 2*c2, 2*c2+1 transposed, so projection work
                    # is available almost immediately as tiles land.
                    wj, b_sb, oT = ((wjq0, bq_sb, qk_next[0]),
                                    (wjk0, bk_sb, qk_next[1]))[w]
                    ps = auxps.tile([P, 256], F32, tag="aux",
                                    name=f"qk0_{w}_{c2}")
                    for hi in range(HT):
                        nc.tensor.matmul(
                            ps,
                            wj[:, hi, :],
                            xT[:, hi, c2 * 256:(c2 + 1) * 256],
                            start=(hi == 0),
                            stop=(hi == HT - 1),
                        )
                    nc.vector.tensor_tensor(
                        oT[:, c2 * 256:(c2 + 1) * 256],
                        ps,
                        b_sb[:, 0:1].to_broadcast((P, 256)),
                        ALU.add,
                    )

                # --- PE-transpose x into [h, s] bf16 layout, interleaved
                # with head-0 q/k projection blocks as seq tiles land ---
                for st in range(ST):
                    xt = xts[st]
                    for jg in range(2):
                        ps = g2ps.tile([P, 4, P], F32, tag="g2")
                        for j4 in range(4):
                            j = jg * 4 + j4
                            nc.tensor.transpose(
                                ps[:, j4, :], xt[:, j * P:(j + 1) * P], ident
                            )
                        # copy on ACT (idle until the first exp ~40us in);
                        # DVE is the chase-phase straggler otherwise
                        nc.scalar.copy(
                            xT[:, jg * 4:(jg + 1) * 4, st * P:(st + 1) * P],
                            ps[:],
                        )
                    if st == 2:
                        # PE-transpose the flat bias rows into [128, 8]
                        for bf, bsb in ((bq_f, bq_sb), (bk_f, bk_sb)):
                            ps8 = g2ps.tile([P, HT], F32, tag="g2",
                                            name="bias8")
                            nc.tensor.transpose(ps8, bf, ident[0:HT, 0:HT])
                            nc.vector.tensor_copy(bsb, ps8)
                    # q/k block c2 covers x tiles 2c2..2c2+1; running one
                    # tile behind the transposes hides the psum->sbuf copy
                    # latency in the dependency chain
                    if st >= 3 and st % 2 == 1:
                        qk0_block256(0, (st - 3) // 2)
                        qk0_block256(1, (st - 3) // 2)
                for c2 in (7,):
                    qk0_block256(0, c2)
                    qk0_block256(1, c2)

                # --- V first half (heads 0-3) ---
                for _ in produce_v_half(0, range(ST)):
                    pass

            with (
                tc.tile_pool(name="pt", bufs=2) as ptpool,
                tc.tile_pool(name="tree", bufs=1) as treepool,
                tc.tile_pool(name="rc", bufs=2) as rcpool,
            ):
                aux_q = deque()
                aux_q.append(("v1", produce_v_half(1, range(ST))))

                def force_drain(key):
                    for k, g in list(aux_q):
                        if k == key:
                            for _ in g:
                                pass
                            aux_q.remove((k, g))

                def emit_gelu(hh, lo, hi):
                    # gelu (sigmoid approx) of head hh, cols [lo,hi). an
                    # holds attn/2, so s = an'*(1 + tanh(1.7018*an')) ==
                    # gelu(attn).
                    nc.scalar.activation(wg[:, lo:hi], an[:, hh, lo:hi],
                                         AF.Tanh, scale=TANH_SCALE)
                    nc.vector.tensor_tensor(wg[:, lo:hi], an[:, hh, lo:hi],
                                            wg[:, lo:hi], ALU.mult)
                    nc.vector.tensor_tensor(an[:, hh, lo:hi],
                                            an[:, hh, lo:hi], wg[:, lo:hi],
                                            ALU.add)

                def emit_cls(hh, qb2):
                    # classify partial of head hh for one 512-col block,
                    # accumulated into clacc; head 7 streams the finished
                    # block straight out to DRAM.
                    lps = auxps.tile([NC, 512], F32, tag="aux",
                                     name=f"cls{hh}_{qb2}")
                    nc.tensor.matmul(
                        lps,
                        wc_sb[:, hh, :],
                        an[:, hh, qb2 * 512:(qb2 + 1) * 512],
                        start=True, stop=True,
                    )
                    cl = clacc[:, qb2 * 512:(qb2 + 1) * 512]
                    if hh == 0:
                        nc.vector.tensor_tensor(
                            cl, lps, bc_sb.to_broadcast((NC, 512)), ALU.add)
                    else:
                        nc.vector.tensor_tensor(cl, cl, lps, ALU.add)
                    if hh == NH - 1:
                        nc.sync.dma_start(
                            out[:, qb2 * 512:(qb2 + 1) * 512], cl)

                # pipeline state of the previous (head, qb) block
                prev = None  # (h, qb, PT, t1)

                def window(h, qb):
                    nonlocal prev
                    # the very last block uses a finer-grained tree so the
                    # end-of-kernel denominator chain after the last exp is
                    # ~2us shorter
                    fine = (h == NH - 1 and qb == SB - 1)
                    qh, kh = qk_cur
                    PT = ptpool.tile([P, ST, 512], BF16, tag="pt")
                    if prev is not None:
                        ph, pqb, pPT, pt1 = prev
                        pv = pvpool.tile([P, 512], F32, tag="pv")
                        rcb = rcpool.tile([P, 512], F32, tag="rc")
                    tA = tB = tC = tD = None
                    qs = qh[:, qb * 512:(qb + 1) * 512]
                    for kg in range(8):
                        ps = g2ps.tile([P, 2, 512], F32, tag="g2")
                        for k2 in range(2):
                            kt = kg * 2 + k2
                            nc.tensor.matmul(
                                ps[:, k2, :],
                                kh[:, kt * P:(kt + 1) * P],
                                qs,
                                start=True,
                                stop=True,
                            )
                        nc.scalar.activation(
                            PT[:, kg * 2:kg * 2 + 2, :], ps[:], AF.Exp,
                            scale=SCALE,
                        )
                        if prev is not None:
                            # two PV matmuls of the previous block (four in
                            # the fine window, so an-mult/gelu/classify of
                            # the second-to-last block can retire early
                            # instead of serializing after the last exp)
                            n = 4 if fine else 2
                            for k2 in range(n):
                                kt = kg * n + k2
                                if kt >= ST:
                                    break
                                nc.tensor.matmul(
                                    pv,
                                    v_sb[:, kt, ph * DK:(ph + 1) * DK],
                                    pPT[:, kt, :],
                                    start=(kt == 0),
                                    stop=(kt == ST - 1),
                                )
                        if fine and prev is not None and kg == 4:
                            nc.vector.tensor_tensor(
                                an[:, ph, pqb * 512:(pqb + 1) * 512],
                                pv, rcb, ALU.mult)
                        if fine and prev is not None and kg == 5:
                            emit_gelu(ph, pqb * 512, (pqb + 1) * 512)
                        if kg == 3:
                            tA = treepool.tile([P, 4, 512], BF16, tag="tA")
                            nc.vector.tensor_tensor(
                                tA, PT[:, 0:4, :], PT[:, 4:8, :], ALU.add)
                            if prev is not None:
                                denb = auxps.tile([P, 512], F32, tag="aux",
                                                  name=f"dnb{ph}_{pqb}")
                                nc.tensor.matmul(denb, twos128, pt1,
                                                 start=True, stop=True)
                                nc.vector.reciprocal_approx_fast(rcb, denb)
                        elif fine and 4 <= kg <= 6:
                            # kg==7's pair is deferred past an-mult(prev) so
                            # the strict DVE FIFO doesn't make an-mult (and
                            # with it the flush PV) wait on the last exp
                            if kg == 4:
                                tB = treepool.tile([P, 4, 512], BF16,
                                                   tag="tB")
                                tC = treepool.tile([P, 4, 512], BF16,
                                                   tag="tC")
                                tD = treepool.tile([P, 2, 512], BF16,
                                                   tag="tD")
                                nc.vector.tensor_tensor(
                                    tC[:, 0:2, :], tA[:, 0:2, :],
                                    tA[:, 2:4, :], ALU.add)
                            if kg == 5:
                                nc.vector.tensor_tensor(
                                    tC[:, 2, :], tC[:, 0, :], tC[:, 1, :],
                                    ALU.add)
                            nc.vector.tensor_tensor(
                                tB[:, kg - 4, :], PT[:, 2 * kg, :],
                                PT[:, 2 * kg + 1, :], ALU.add)
                            if kg == 6:
                                nc.vector.tensor_tensor(
                                    tD[:, 0, :], tB[:, 0, :], tB[:, 1, :],
                                    ALU.add)
                        # ration aux work (~2 blocks per window) so the
                        # supply lasts through the whole head instead of
                        # front-loading and starving later windows
                        if kg in (1, 5):
                            while aux_q:
                                try:
                                    next(aux_q[0][1])
                                    aux_q.rotate(-1)
                                    break
                                except StopIteration:
                                    aux_q.popleft()
                    # finish previous block: normalize (an' = attn/2 since
                    # rcb = 1/(2*denom))
                    if prev is not None and not fine:
                        nc.vector.tensor_tensor(
                            an[:, ph, pqb * 512:(pqb + 1) * 512],
                            pv, rcb, ALU.mult)
                    # fused denominator tree tail for current block
                    t1 = treepool.tile([P, 512], BF16, tag="tE")
                    if fine:
                        nc.vector.tensor_tensor(
                            tB[:, 3, :], PT[:, 14, :], PT[:, 15, :], ALU.add)
                        nc.vector.tensor_tensor(
                            tD[:, 1, :], tB[:, 2, :], tB[:, 3, :], ALU.add)
                        nc.vector.tensor_tensor(
                            tC[:, 3, :], tD[:, 0, :], tD[:, 1, :], ALU.add)
                        nc.vector.tensor_tensor(
                            t1, tC[:, 2, :], tC[:, 3, :], ALU.add)
                    else:
                        tB = treepool.tile([P, 4, 512], BF16, tag="tB")
                        nc.vector.tensor_tensor(
                            tB, PT[:, 8:12, :], PT[:, 12:16, :], ALU.add)
                        tC = treepool.tile([P, 4, 512], BF16, tag="tC")
                        nc.vector.tensor_tensor(tC, tA, tB, ALU.add)
                        tD = treepool.tile([P, 2, 512], BF16, tag="tD")
                        nc.vector.tensor_tensor(tD, tC[:, 0:2, :],
                                                tC[:, 2:4, :], ALU.add)
                        nc.vector.tensor_tensor(t1, tD[:, 0, :], tD[:, 1, :],
                                                ALU.add)
                    prev = (h, qb, PT, t1)

                qk_cur = qk_next
                gen_qk7b = None
                for h in range(NH):
                    force_drain(f"qk{h}")
                    if h == NH - 1 and gen_qk7b is not None:
                        # queue the deferred head-7 q blocks: they fill the
                        # aux-starved head-7 windows (block ss is pulled two
                        # windows before window (7, ss) consumes it)
                        aux_q.append(("qk7b", gen_qk7b))
                    if h == 4:
                        force_drain("v1")
                    for qb in range(SB):
                        if qb == 1 and h >= 1:
                            emit_gelu(h - 1, 0, S)
                        window(h, qb)
                        if qb == 2 and h >= 1:
                            for qb2 in range(SB):
                                emit_cls(h - 1, qb2)
                        if h == NH - 1 and qb >= 1:
                            # head-7 epilogue per block, staggered so the
                            # tanh/DVE chain hides in the aux-starved last
                            # head's windows instead of serializing the tail
                            # (blocks 2 and 3 are handled by the fine last
                            # window and the flush)
                            if qb <= 2:
                                emit_gelu(h, (qb - 1) * 512, qb * 512)
                            if qb >= 2:
                                emit_cls(h, qb - 2)
                        if qb == 0 and h + 1 < NH:
                            qk_next = alloc_qk(h + 1)
                            if h + 1 == NH - 1:
                                # head 7: produce k (+ q block 0) during head
                                # 6 as usual, but defer q blocks 1-3 into
                                # head 7's own (aux-starved) windows
                                t7 = qk_next
                                wjq7 = wjpool.tile([P, HT, P], BF16,
                                                   tag="wj", name="wj7q")
                                nc.gpsimd.dma_start(
                                    wjq7, wq_r[:, :, (NH - 1) * P:NH * P])
                                wjk7 = wjpool.tile([P, HT, P], BF16,
                                                   tag="wj", name="wj7k")
                                nc.gpsimd.dma_start(
                                    wjk7, wk_r[:, :, (NH - 1) * P:NH * P])

                                def gen_a():
                                    for ss in range(4):
                                        qk_block(NH - 1, wjk7, bk_sb,
                                                 t7[1], ss)
                                        yield
                                    qk_block(NH - 1, wjq7, bq_sb, t7[0], 0)
                                    yield

                                def gen_b():
                                    for ss in (1, 2, 3):
                                        qk_block(NH - 1, wjq7, bq_sb,
                                                 t7[0], ss)
                                        yield

                                aux_q.append((f"qk{h + 1}", gen_a()))
                                gen_qk7b = gen_b()
                            else:
                                aux_q.append(
                                    (f"qk{h + 1}",
                                     produce_qk(h + 1, qk_next))
                                )
                        if h == NH - 1 and qb == 2:
                            force_drain("qk7b")
                    qk_cur = qk_next

                # --- flush: PV + normalize of the last block, then the
                # remaining head-7 epilogue blocks ---
                ph, pqb, pPT, pt1 = prev
                pv = pvpool.tile([P, 512], F32, tag="pv")
                rcb = rcpool.tile([P, 512], F32, tag="rc")
                denb = auxps.tile([P, 512], F32, tag="aux", name="dnb_last")
                # PV first: its early chunks only need early exp groups, so
                # it overlaps the denominator tree/reciprocal of the last
                # block instead of idling behind them
                for kt in range(ST):
                    nc.tensor.matmul(
                        pv,
                        v_sb[:, kt, ph * DK:(ph + 1) * DK],
                        pPT[:, kt, :],
                        start=(kt == 0),
                        stop=(kt == ST - 1),
                    )
                    if kt == 7:
                        emit_cls(NH - 1, 2)
                    if kt == 11:
                        nc.tensor.matmul(denb, twos128, pt1,
                                         start=True, stop=True)
                        nc.vector.reciprocal_approx_fast(rcb, denb)
                # final block: normalize + gelu in halves so the ACT/DVE
                # chain pipelines instead of serializing 512-wide ops
                for hf in range(2):
                    sl0 = pqb * 512 + hf * 256
                    nc.vector.tensor_tensor(
                        an[:, ph, sl0:sl0 + 256], pv[:, hf * 256:(hf + 1) * 256],
                        rcb[:, hf * 256:(hf + 1) * 256], ALU.mult)
                    emit_gelu(NH - 1, sl0, sl0 + 256)
                emit_cls(NH - 1, 3)

    nc.finalize()
    return nc


def get_nc():
    if not _NC_CACHE:
        _NC_CACHE.append(_build())
    return _NC_CACHE[0]


def kernel(**inputs) -> np.ndarray:
    ins = {k: np.ascontiguousarray(np.asarray(v, dtype=np.float32))
           for k, v in inputs.items()}
    x = ins["x"]
    assert x.shape == (B, S, H), x.shape
    shared = {k: ins[k] for k in
              ("Wq", "bq", "Wk", "bk", "Wv", "bv", "Wc", "bc")}
    in_maps = [{"x": x[b], **shared} for b in range(B)]
    nc = get_nc()
    res = run_bass_kernel_spmd(nc, in_maps, core_ids=list(range(B)))
    outs = [np.asarray(res.results[b]["out"], dtype=np.float32).T
            for b in range(B)]
    return np.stack(outs, axis=0)


# revision 45
# speedup vs baseline: 1.0020x; 1.0020x over previous
"""TRN2 Bass kernel for nn_CustomHeadMultiHeadAttention (dense transformer).

Full inputs: x [8, 2048, 1024] f32 + QKV/classify weights. Sharding: pure
data parallelism — batch 8 across 8 NeuronCores, one batch element per core.
Each core runs the complete MHA + GELU + classify on its slice; no
collectives. Host only slices the batch and stacks/transposes the outputs.

Per-core pipeline (bf16 matmul operands, fp32 PSUM accumulation), v2:
  startup: PE-transposes of x seq-tiles, q0/k0 projection blocks and the
    first half of V are interleaved so the PE chases the x DMA stream
    instead of FIFO-stalling on the last tile.
  per head h, per 512-wide q block (software-pipelined one block deep):
    scores^T tiles = kh-chunk (lhsT) @ qh      [k=128, q=512] x2 per group
    P^T  = exp(scores^T / sqrt(dk))            ACT PSUM->SBUF bf16
    PV of the PREVIOUS block is interleaved between score groups, so the
    PE never head-of-line blocks on the exp stream.
    denom = 5-op fused bf16 tree over P^T + twos[128,128] @ t1 broadcast
    (the 2x folds the 0.5 of gelu into the softmax reciprocal)
    an'  = attn^T/2 = pv * reciprocal_approx_fast(2*denom)
  per head epilogue (no ACT table switches: exp/tanh/copy share one set):
    gelu_sigmoid: t = tanh(1.7018*an'); s = an' + an'*t  == gelu(attn)
    classify partial: clacc[2, S] += Wc-chunk(h) lhsT @ s   (DVE accum)
  out = clacc + bc (accumulated from head 0), single DMA [2, S].
Host transposes logits^T [2, s] -> [2048, 2].
"""

import math
import sys
from collections import deque

sys.path.insert(0, "/opt/trn_rl_repo")

import numpy as np

import concourse.bass as bass
import concourse.mybir as mybir
import concourse.tile as tile
from concourse import bacc
from concourse.bass_utils import run_bass_kernel_spmd
from concourse.masks import make_identity

AF = mybir.ActivationFunctionType
ALU = mybir.AluOpType
F32 = mybir.dt.float32
BF16 = mybir.dt.bfloat16

B = 8           # batch (== number of cores)
S = 2048        # sequence length
H = 1024        # hidden
NH = 8          # heads
DK = 128        # head dim
P = 128         # partitions
NC = 2          # classes
SB = S // 512   # 4 q/s blocks of 512
HT = H // P     # 8 hidden tiles
ST = S // P     # 16 seq tiles
SCALE = 1.0 / math.sqrt(DK)
TANH_SCALE = 1.7018  # tanh arg on an' = attn/2: tanh(0.8509 * attn)

_NC_CACHE = []


def _build():
    nc = bacc.Bacc(None, target_bir_lowering=False, debug=False)

    x = nc.dram_tensor("x", [S, H], F32, kind="ExternalInput")
    Wq = nc.dram_tensor("Wq", [H, H], F32, kind="ExternalInput")
    bq = nc.dram_tensor("bq", [H], F32, kind="ExternalInput")
    Wk = nc.dram_tensor("Wk", [H, H], F32, kind="ExternalInput")
    bk = nc.dram_tensor("bk", [H], F32, kind="ExternalInput")
    Wv = nc.dram_tensor("Wv", [H, H], F32, kind="ExternalInput")
    bv = nc.dram_tensor("bv", [H], F32, kind="ExternalInput")
    Wc = nc.dram_tensor("Wc", [H, NC], F32, kind="ExternalInput")
    bc = nc.dram_tensor("bc", [NC], F32, kind="ExternalInput")
    out = nc.dram_tensor("out", [NC, S], F32, kind="ExternalOutput")

    with tile.TileContext(nc) as tc:
        with (
            tc.tile_pool(name="persist", bufs=1) as persist,
            tc.tile_pool(name="g2ps", bufs=2, space="PSUM") as g2ps,
            tc.tile_pool(name="pvps", bufs=2, space="PSUM") as pvpool,
            tc.tile_pool(name="auxps", bufs=2, space="PSUM") as auxps,
            tc.tile_pool(name="wj", bufs=3) as wjpool,
            tc.tile_pool(name="qk", bufs=2) as qkpool,
        ):
            ident = persist.tile([P, P], F32, tag="ident")
            make_identity(nc, ident)
            twos128 = persist.tile([P, P], BF16, tag="twos128")
            nc.vector.memset(twos128, 2.0)

            xT = persist.tile([P, HT, S], BF16, tag="xT")
            wv_sb = persist.tile([P, HT, H], BF16, tag="wv")
            v_sb = persist.tile([P, ST, H], BF16, tag="v")
            an = persist.tile([P, NH, S], BF16, tag="an")
            wg = persist.tile([P, S], BF16, tag="wg")
            clacc = persist.tile([NC, S], F32, tag="clacc")
            wq_r = Wq.rearrange("(o p) d -> p o d", p=P)
            wk_r = Wk.rearrange("(o p) d -> p o d", p=P)
            wv_r = Wv.rearrange("(o p) d -> p o d", p=P)

            bq_sb = persist.tile([P, HT], F32, tag="bq")
            bk_sb = persist.tile([P, HT], F32, tag="bk")
            bv_bc = persist.tile([P, H], BF16, tag="bv")
            bc_sb = persist.tile([NC, 1], F32, tag="bc")
            wc_sb = persist.tile([P, HT, NC], BF16, tag="wc")

            def alloc_qk(h):
                qh = qkpool.tile([P, S], BF16, tag="qh", name=f"qh{h}")
                kh = qkpool.tile([P, S], BF16, tag="kh", name=f"kh{h}")
                return qh, kh

            def qk_block(h, wj, b_sb, oT, ss):
                ps = auxps.tile([P, 512], F32, tag="aux",
                                name=f"qk{h}_{ss}")
                for hi in range(HT):
                    nc.tensor.matmul(
                        ps,
                        wj[:, hi, :],
                        xT[:, hi, ss * 512:(ss + 1) * 512],
                        start=(hi == 0),
                        stop=(hi == HT - 1),
                    )
                nc.vector.tensor_tensor(
                    oT[:, ss * 512:(ss + 1) * 512],
                    ps,
                    b_sb[:, h:h + 1].to_broadcast((P, 512)),
                    ALU.add,
                )

            def produce_qk(h, tiles):
                """Project q,k for head h; yields per (w, ss) 8-MM block."""
                qh, kh = tiles
                for w_r, b_sb, oT in ((wq_r, bq_sb, qh), (wk_r, bk_sb, kh)):
                    wj = wjpool.tile([P, HT, P], BF16, tag="wj",
                                     name=f"wj{h}")
                    nc.gpsimd.dma_start(wj, w_r[:, :, h * P:(h + 1) * P])
                    for ss in range(4):
                        qk_block(h, wj, b_sb, oT, ss)
                        yield

            def produce_v_half(dh, sts):
                for st in sts:
                    ps = auxps.tile([P, 512], F32, tag="aux",
                                    name=f"v{dh}_{st}")
                    for hi in range(HT):
                        nc.tensor.matmul(
                            ps,
                            xT[:, hi, st * P:(st + 1) * P],
                            wv_sb[:, hi, dh * 512:(dh + 1) * 512],
                            start=(hi == 0),
                            stop=(hi == HT - 1),
                        )
                    nc.vector.tensor_tensor(
                        v_sb[:, st, dh * 512:(dh + 1) * 512],
                        ps,
                        bv_bc[:, dh * 512:(dh + 1) * 512],
                        ALU.add,
                    )
                    yield

            with tc.tile_pool(name="xload", bufs=1) as xload:
                # gpsimd (casting SWDGE) queue order: q0/k0 weights first so
                # the head-0 projection can start early; V's first half next.
                qk_next = alloc_qk(0)
                # flat bias rows first on the gpsimd ring (tiny, ~10KB;
                # they gate the PE bias-transpose at st==4); then the
                # head-0 q/k weights (the SWDGE casting path runs at only
                # ~22GB/s while the x stream owns HBM, so these 256KB
                # blocks land ~16-18us in — the chase is ordered around
                # that); then V's first half.
                bq_f = xload.tile([HT, P], F32, tag="bqf")
                bk_f = xload.tile([HT, P], F32, tag="bkf")
                nc.gpsimd.dma_start(bq_f, bq.rearrange("(j p) -> j p", p=P))
                nc.gpsimd.dma_start(bk_f, bk.rearrange("(j p) -> j p", p=P))
                nc.gpsimd.dma_start(bc_sb, bc[:, None])
                wjq0 = wjpool.tile([P, HT, P], BF16, tag="wj", name="wj0q")
                nc.gpsimd.dma_start(wjq0, wq_r[:, :, 0:P])
                wjk0 = wjpool.tile([P, HT, P], BF16, tag="wj", name="wj0k")
                nc.gpsimd.dma_start(wjk0, wk_r[:, :, 0:P])
                for hi in range(HT):
                    nc.gpsimd.dma_start(wv_sb[:, hi, 0:512],
                                        wv_r[:, hi, 0:512])
                nc.gpsimd.dma_start(bv_bc, bv[None, :].to_broadcast((P, H)))
                for hi in range(HT):
                    nc.gpsimd.dma_start(wv_sb[:, hi, 512:1024],
                                        wv_r[:, hi, 512:1024])
                nc.gpsimd.dma_start(wc_sb, Wc.rearrange("(j p) c -> p j c", p=P))

                # sync (fast HWDGE) queue: nothing but the x stream — the
                # first tiles gate the PE
                xts = []
                for st in range(ST):
                    xt = xload.tile([P, H], F32, tag=f"xl{st % 8}",
                                    name=f"xt{st}")
                    nc.sync.dma_start(xt, x[st * P:(st + 1) * P, :])
                    xts.append(xt)

                def qk0_block256(w, c2):
                    # head-0 q/k in 256-wide blocks: block c2 needs only x
                    # seq tiles 2*c2, 2*c2+1 transposed, so projection work
                    # is available almost immediately as tiles land.
                    wj, b_sb, oT = ((wjq0, bq_sb, qk_next[0]),
                                    (wjk0, bk_sb, qk_next[1]))[w]
                    ps = auxps.tile([P, 256], F32, tag="aux",
                                    name=f"qk0_{w}_{c2}")
                    for hi in range(HT):
                        nc.tensor.matmul(
                            ps,
                            wj[:, hi, :],
                            xT[:, hi, c2 * 256:(c2 + 1) * 256],
                            start=(hi == 0),
                            stop=(hi == HT - 1),
                        )
                    nc.vector.tensor_tensor(
                        oT[:, c2 * 256:(c2 + 1) * 256],
                        ps,
                        b_sb[:, 0:1].to_broadcast((P, 256)),
                        ALU.add,
                    )

                # --- PE-transpose x into [h, s] bf16 layout, interleaved
                # with head-0 q/k projection blocks as seq tiles land ---
                for st in range(ST):
                    xt = xts[st]
                    for jg in range(2):
                        ps = g2ps.tile([P, 4, P], F32, tag="g2")
                        for j4 in range(4):
                            j = jg * 4 + j4
                            nc.tensor.transpose(
                                ps[:, j4, :], xt[:, j * P:(j + 1) * P], ident
                            )
                        # copy on ACT (idle until the first exp ~40us in);
                        # DVE is the chase-phase straggler otherwise
                        nc.scalar.copy(
                            xT[:, jg * 4:(jg + 1) * 4, st * P:(st + 1) * P],
                            ps[:],
                        )
                    if st == 4:
                        # PE-transpose the flat bias rows into [128, 8]
                        for bf, bsb in ((bq_f, bq_sb), (bk_f, bk_sb)):
                            ps8 = g2ps.tile([P, HT], F32, tag="g2",
                                            name="bias8")
                            nc.tensor.transpose(ps8, bf, ident[0:HT, 0:HT])
                            nc.vector.tensor_copy(bsb, ps8)
                    # q/k blocks start once wjq0's slow casting DMA has
                    # landed (~st 7); block c2 only needs x tiles <= 2c2+1,
                    # all transposed+copied well before
                    if st >= 7 and st % 2 == 1:
                        c2 = (st - 7) // 2
                        qk0_block256(0, c2)
                        qk0_block256(1, c2)
                for c2 in (5, 6, 7):
                    qk0_block256(0, c2)
                    qk0_block256(1, c2)

                # --- V first half (heads 0-3) ---
                for _ in produce_v_half(0, range(ST)):
                    pass

            with (
                tc.tile_pool(name="pt", bufs=2) as ptpool,
                tc.tile_pool(name="tree", bufs=1) as treepool,
                tc.tile_pool(name="rc", bufs=2) as rcpool,
            ):
                aux_q = deque()
                aux_q.append(("v1", produce_v_half(1, range(ST))))

                def force_drain(key):
                    for k, g in list(aux_q):
                        if k == key:
                            for _ in g:
                                pass
                            aux_q.remove((k, g))

                def emit_gelu(hh, lo, hi):
                    # gelu (sigmoid approx) of head hh, cols [lo,hi). an
                    # holds attn/2, so s = an'*(1 + tanh(1.7018*an')) ==
                    # gelu(attn).
                    nc.scalar.activation(wg[:, lo:hi], an[:, hh, lo:hi],
                                         AF.Tanh, scale=TANH_SCALE)
                    nc.vector.tensor_tensor(wg[:, lo:hi], an[:, hh, lo:hi],
                                            wg[:, lo:hi], ALU.mult)
                    nc.vector.tensor_tensor(an[:, hh, lo:hi],
                                            an[:, hh, lo:hi], wg[:, lo:hi],
                                            ALU.add)

                def emit_cls(hh, qb2):
                    # classify partial of head hh for one 512-col block,
                    # accumulated into clacc; head 7 streams the finished
                    # block straight out to DRAM.
                    lps = auxps.tile([NC, 512], F32, tag="aux",
                                     name=f"cls{hh}_{qb2}")
                    nc.tensor.matmul(
                        lps,
                        wc_sb[:, hh, :],
                        an[:, hh, qb2 * 512:(qb2 + 1) * 512],
                        start=True, stop=True,
                    )
                    cl = clacc[:, qb2 * 512:(qb2 + 1) * 512]
                    if hh == 0:
                        nc.vector.tensor_tensor(
                            cl, lps, bc_sb.to_broadcast((NC, 512)), ALU.add)
                    else:
                        nc.vector.tensor_tensor(cl, cl, lps, ALU.add)
                    if hh == NH - 1:
                        nc.sync.dma_start(
                            out[:, qb2 * 512:(qb2 + 1) * 512], cl)

                # pipeline state of the previous (head, qb) block
                prev = None  # (h, qb, PT, t1)

                def window(h, qb):
                    nonlocal prev
                    # the very last block uses a finer-grained tree so the
                    # end-of-kernel denominator chain after the last exp is
                    # ~2us shorter
                    fine = (h == NH - 1 and qb == SB - 1)
                    qh, kh = qk_cur
                    PT = ptpool.tile([P, ST, 512], BF16, tag="pt")
                    if prev is not None:
                        ph, pqb, pPT, pt1 = prev
                        pv = pvpool.tile([P, 512], F32, tag="pv")
                        rcb = rcpool.tile([P, 512], F32, tag="rc")
                    tA = tB = tC = tD = None
                    qs = qh[:, qb * 512:(qb + 1) * 512]
                    for kg in range(8):
                        ps = g2ps.tile([P, 2, 512], F32, tag="g2")
                        for k2 in range(2):
                            kt = kg * 2 + k2
                            nc.tensor.matmul(
                                ps[:, k2, :],
                                kh[:, kt * P:(kt + 1) * P],
                                qs,
                                start=True,
                                stop=True,
                            )
                        if fine:
                            # per-kt exps: the last window's denominator
                            # tree chases exp completion at half granularity
                            for k2 in range(2):
                                nc.scalar.activation(
                                    PT[:, kg * 2 + k2, :], ps[:, k2, :],
                                    AF.Exp, scale=SCALE,
                                )
                        else:
                            nc.scalar.activation(
                                PT[:, kg * 2:kg * 2 + 2, :], ps[:], AF.Exp,
                                scale=SCALE,
                            )
                        if prev is not None:
                            # two PV matmuls of the previous block (four in
                            # the fine window, so an-mult/gelu/classify of
                            # the second-to-last block can retire early
                            # instead of serializing after the last exp)
                            n = 4 if fine else 2
                            for k2 in range(n):
                                kt = kg * n + k2
                                if kt >= ST:
                                    break
                                nc.tensor.matmul(
                                    pv,
                                    v_sb[:, kt, ph * DK:(ph + 1) * DK],
                                    pPT[:, kt, :],
                                    start=(kt == 0),
                                    stop=(kt == ST - 1),
                                )
                        if fine and prev is not None and kg == 4:
                            nc.vector.tensor_tensor(
                                an[:, ph, pqb * 512:(pqb + 1) * 512],
                                pv, rcb, ALU.mult)
                        if fine and prev is not None and kg == 5:
                            emit_gelu(ph, pqb * 512, (pqb + 1) * 512)
                        if kg == 3:
                            tA = treepool.tile([P, 4, 512], BF16, tag="tA")
                            nc.vector.tensor_tensor(
                                tA, PT[:, 0:4, :], PT[:, 4:8, :], ALU.add)
                            if prev is not None:
                                denb = auxps.tile([P, 512], F32, tag="aux",
                                                  name=f"dnb{ph}_{pqb}")
                                nc.tensor.matmul(denb, twos128, pt1,
                                                 start=True, stop=True)
                                nc.vector.reciprocal_approx_fast(rcb, denb)
                        elif fine and 4 <= kg <= 6:
                            # kg==7's pair is deferred past an-mult(prev) so
                            # the strict DVE FIFO doesn't make an-mult (and
                            # with it the flush PV) wait on the last exp
                            if kg == 4:
                                tB = treepool.tile([P, 4, 512], BF16,
                                                   tag="tB")
                                tC = treepool.tile([P, 4, 512], BF16,
                                                   tag="tC")
                                tD = treepool.tile([P, 2, 512], BF16,
                                                   tag="tD")
                                nc.vector.tensor_tensor(
                                    tC[:, 0:2, :], tA[:, 0:2, :],
                                    tA[:, 2:4, :], ALU.add)
                            if kg == 5:
                                nc.vector.tensor_tensor(
                                    tC[:, 2, :], tC[:, 0, :], tC[:, 1, :],
                                    ALU.add)
                            nc.vector.tensor_tensor(
                                tB[:, kg - 4, :], PT[:, 2 * kg, :],
                                PT[:, 2 * kg + 1, :], ALU.add)
                            if kg == 6:
                                nc.vector.tensor_tensor(
                                    tD[:, 0, :], tB[:, 0, :], tB[:, 1, :],
                                    ALU.add)
                        # ration aux work (~2 blocks per window) so the
                        # supply lasts through the whole head instead of
                        # front-loading and starving later windows
                        if kg in (1, 5):
                            while aux_q:
                                try:
                                    next(aux_q[0][1])
                                    aux_q.rotate(-1)
                                    break
                                except StopIteration:
                                    aux_q.popleft()
                    # finish previous block: normalize (an' = attn/2 since
                    # rcb = 1/(2*denom))
                    if prev is not None and not fine:
                        nc.vector.tensor_tensor(
                            an[:, ph, pqb * 512:(pqb + 1) * 512],
                            pv, rcb, ALU.mult)
                    # fused denominator tree tail for current block
                    t1 = treepool.tile([P, 512], BF16, tag="tE")
                    if fine:
                        nc.vector.tensor_tensor(
                            tB[:, 3, :], PT[:, 14, :], PT[:, 15, :], ALU.add)
                        nc.vector.tensor_tensor(
                            tD[:, 1, :], tB[:, 2, :], tB[:, 3, :], ALU.add)
                        nc.vector.tensor_tensor(
                            tC[:, 3, :], tD[:, 0, :], tD[:, 1, :], ALU.add)
                        nc.vector.tensor_tensor(
                            t1, tC[:, 2, :], tC[:, 3, :], ALU.add)
                    else:
                        tB = treepool.tile([P, 4, 512], BF16, tag="tB")
                        nc.vector.tensor_tensor(
                            tB, PT[:, 8:12, :], PT[:, 12:16, :], ALU.add)
                        tC = treepool.tile([P, 4, 512], BF16, tag="tC")
                        nc.vector.tensor_tensor(tC, tA, tB, ALU.add)
                        tD = treepool.tile([P, 2, 512], BF16, tag="tD")
                        nc.vector.tensor_tensor(tD, tC[:, 0:2, :],
                                                tC[:, 2:4, :], ALU.add)
                        nc.vector.tensor_tensor(t1, tD[:, 0, :], tD[:, 1, :],
                                                ALU.add)
                    prev = (h, qb, PT, t1)

                qk_cur = qk_next
                gen_qk7b = None
                for h in range(NH):
                    force_drain(f"qk{h}")
                    if h == NH - 1 and gen_qk7b is not None:
                        # queue the deferred head-7 q blocks: they fill the
                        # aux-starved head-7 windows (block ss is pulled two
                        # windows before window (7, ss) consumes it)
                        aux_q.append(("qk7b", gen_qk7b))
                    if h == 4:
                        force_drain("v1")
                    for qb in range(SB):
                        if qb == 1 and h >= 1:
                            emit_gelu(h - 1, 0, S)
                        window(h, qb)
                        if qb == 2 and h >= 1:
                            for qb2 in range(SB):
                                emit_cls(h - 1, qb2)
                        if h == NH - 1 and qb >= 1:
                            # head-7 epilogue per block, staggered so the
                            # tanh/DVE chain hides in the aux-starved last
                            # head's windows instead of serializing the tail
                            # (blocks 2 and 3 are handled by the fine last
                            # window and the flush)
                            if qb <= 2:
                                emit_gelu(h, (qb - 1) * 512, qb * 512)
                            if qb >= 2:
                                emit_cls(h, qb - 2)
                        if qb == 0 and h + 1 < NH:
                            qk_next = alloc_qk(h + 1)
                            if h + 1 == NH - 1:
                                # head 7: produce k (+ q block 0) during head
                                # 6 as usual, but defer q blocks 1-3 into
                                # head 7's own (aux-starved) windows
                                t7 = qk_next
                                wjq7 = wjpool.tile([P, HT, P], BF16,
                                                   tag="wj", name="wj7q")
                                nc.gpsimd.dma_start(
                                    wjq7, wq_r[:, :, (NH - 1) * P:NH * P])
                                wjk7 = wjpool.tile([P, HT, P], BF16,
                                                   tag="wj", name="wj7k")
                                nc.gpsimd.dma_start(
                                    wjk7, wk_r[:, :, (NH - 1) * P:NH * P])

                                def gen_a():
                                    for ss in range(4):
                                        qk_block(NH - 1, wjk7, bk_sb,
                                                 t7[1], ss)
                                        yield
                                    qk_block(NH - 1, wjq7, bq_sb, t7[0], 0)
                                    yield

                                def gen_b():
                                    for ss in (1, 2, 3):
                                        qk_block(NH - 1, wjq7, bq_sb,
                                                 t7[0], ss)
                                        yield

                                aux_q.append((f"qk{h + 1}", gen_a()))
                                gen_qk7b = gen_b()
                            else:
                                aux_q.append(
                                    (f"qk{h + 1}",
                                     produce_qk(h + 1, qk_next))
                                )
                        if h == NH - 1 and qb == 2:
                            force_drain("qk7b")
                    qk_cur = qk_next

                # --- flush: PV + normalize of the last block, then the
                # remaining head-7 epilogue blocks ---
                ph, pqb, pPT, pt1 = prev
                pv = pvpool.tile([P, 512], F32, tag="pv")
                rcb = rcpool.tile([P, 512], F32, tag="rc")
                denb = auxps.tile([P, 512], F32, tag="aux", name="dnb_last")
                # PV first: its early chunks only need early exp groups, so
                # it overlaps the denominator tree/reciprocal of the last
                # block instead of idling behind them
                for kt in range(ST):
                    nc.tensor.matmul(
                        pv,
                        v_sb[:, kt, ph * DK:(ph + 1) * DK],
                        pPT[:, kt, :],
                        start=(kt == 0),
                        stop=(kt == ST - 1),
                    )
                    if kt == 7:
                        emit_cls(NH - 1, 2)
                    if kt == 11:
                        nc.tensor.matmul(denb, twos128, pt1,
                                         start=True, stop=True)
                        nc.vector.reciprocal_approx_fast(rcb, denb)
                # final block: normalize + gelu in halves so the ACT/DVE
                # chain pipelines instead of serializing 512-wide ops
                for hf in range(2):
                    sl0 = pqb * 512 + hf * 256
                    nc.vector.tensor_tensor(
                        an[:, ph, sl0:sl0 + 256], pv[:, hf * 256:(hf + 1) * 256],
                        rcb[:, hf * 256:(hf + 1) * 256], ALU.mult)
                    emit_gelu(NH - 1, sl0, sl0 + 256)
                emit_cls(NH - 1, 3)

    nc.finalize()
    return nc


def get_nc():
    if not _NC_CACHE:
        _NC_CACHE.append(_build())
    return _NC_CACHE[0]


def kernel(**inputs) -> np.ndarray:
    ins = {k: np.ascontiguousarray(np.asarray(v, dtype=np.float32))
           for k, v in inputs.items()}
    x = ins["x"]
    assert x.shape == (B, S, H), x.shape
    shared = {k: ins[k] for k in
              ("Wq", "bq", "Wk", "bk", "Wv", "bv", "Wc", "bc")}
    in_maps = [{"x": x[b], **shared} for b in range(B)]
    nc = get_nc()
    res = run_bass_kernel_spmd(nc, in_maps, core_ids=list(range(B)))
    outs = [np.asarray(res.results[b]["out"], dtype=np.float32).T
            for b in range(B)]
    return np.stack(outs, axis=0)


# revision 48
# speedup vs baseline: 1.0037x; 1.0017x over previous
"""TRN2 Bass kernel for nn_CustomHeadMultiHeadAttention (dense transformer).

Full inputs: x [8, 2048, 1024] f32 + QKV/classify weights. Sharding: pure
data parallelism — batch 8 across 8 NeuronCores, one batch element per core.
Each core runs the complete MHA + GELU + classify on its slice; no
collectives. Host only slices the batch and stacks/transposes the outputs.

Per-core pipeline (bf16 matmul operands, fp32 PSUM accumulation), v2:
  startup: PE-transposes of x seq-tiles, q0/k0 projection blocks and the
    first half of V are interleaved so the PE chases the x DMA stream
    instead of FIFO-stalling on the last tile.
  per head h, per 512-wide q block (software-pipelined one block deep):
    scores^T tiles = kh-chunk (lhsT) @ qh      [k=128, q=512] x2 per group
    P^T  = exp(scores^T / sqrt(dk))            ACT PSUM->SBUF bf16
    PV of the PREVIOUS block is interleaved between score groups, so the
    PE never head-of-line blocks on the exp stream.
    denom = 5-op fused bf16 tree over P^T + twos[128,128] @ t1 broadcast
    (the 2x folds the 0.5 of gelu into the softmax reciprocal)
    an'  = attn^T/2 = pv * reciprocal_approx_fast(2*denom)
  per head epilogue (no ACT table switches: exp/tanh/copy share one set):
    gelu_sigmoid: t = tanh(1.7018*an'); s = an' + an'*t  == gelu(attn)
    classify partial: clacc[2, S] += Wc-chunk(h) lhsT @ s   (DVE accum)
  out = clacc + bc (accumulated from head 0), single DMA [2, S].
Host transposes logits^T [2, s] -> [2048, 2].
"""

import math
import sys
from collections import deque

sys.path.insert(0, "/opt/trn_rl_repo")

import numpy as np

import concourse.bass as bass
import concourse.mybir as mybir
import concourse.tile as tile
from concourse import bacc
from concourse.bass_utils import run_bass_kernel_spmd
from concourse.masks import make_identity

AF = mybir.ActivationFunctionType
ALU = mybir.AluOpType
F32 = mybir.dt.float32
BF16 = mybir.dt.bfloat16

B = 8           # batch (== number of cores)
S = 2048        # sequence length
H = 1024        # hidden
NH = 8          # heads
DK = 128        # head dim
P = 128         # partitions
NC = 2          # classes
SB = S // 512   # 4 q/s blocks of 512
HT = H // P     # 8 hidden tiles
ST = S // P     # 16 seq tiles
SCALE = 1.0 / math.sqrt(DK)
TANH_SCALE = 1.7018  # tanh arg on an' = attn/2: tanh(0.8509 * attn)

_NC_CACHE = []


def _build():
    nc = bacc.Bacc(None, target_bir_lowering=False, debug=False)

    x = nc.dram_tensor("x", [S, H], F32, kind="ExternalInput")
    Wq = nc.dram_tensor("Wq", [H, H], F32, kind="ExternalInput")
    bq = nc.dram_tensor("bq", [H], F32, kind="ExternalInput")
    Wk = nc.dram_tensor("Wk", [H, H], F32, kind="ExternalInput")
    bk = nc.dram_tensor("bk", [H], F32, kind="ExternalInput")
    Wv = nc.dram_tensor("Wv", [H, H], F32, kind="ExternalInput")
    bv = nc.dram_tensor("bv", [H], F32, kind="ExternalInput")
    Wc = nc.dram_tensor("Wc", [H, NC], F32, kind="ExternalInput")
    bc = nc.dram_tensor("bc", [NC], F32, kind="ExternalInput")
    out = nc.dram_tensor("out", [NC, S], F32, kind="ExternalOutput")

    with tile.TileContext(nc) as tc:
        with (
            tc.tile_pool(name="persist", bufs=1) as persist,
            tc.tile_pool(name="g2ps", bufs=2, space="PSUM") as g2ps,
            tc.tile_pool(name="pvps", bufs=2, space="PSUM") as pvpool,
            tc.tile_pool(name="auxps", bufs=2, space="PSUM") as auxps,
            tc.tile_pool(name="wj", bufs=3) as wjpool,
            tc.tile_pool(name="qk", bufs=2) as qkpool,
        ):
            ident = persist.tile([P, P], F32, tag="ident")
            make_identity(nc, ident)
            twos128 = persist.tile([P, P], BF16, tag="twos128")
            nc.vector.memset(twos128, 2.0)

            xT = persist.tile([P, HT, S], BF16, tag="xT")
            wv_sb = persist.tile([P, HT, H], BF16, tag="wv")
            v_sb = persist.tile([P, ST, H], BF16, tag="v")
            an = persist.tile([P, NH, S], BF16, tag="an")
            wg = persist.tile([P, S], BF16, tag="wg")
            clacc = persist.tile([NC, S], F32, tag="clacc")
            wq_r = Wq.rearrange("(o p) d -> p o d", p=P)
            wk_r = Wk.rearrange("(o p) d -> p o d", p=P)
            wv_r = Wv.rearrange("(o p) d -> p o d", p=P)

            bq_sb = persist.tile([P, HT], F32, tag="bq")
            bk_sb = persist.tile([P, HT], F32, tag="bk")
            bv_bc = persist.tile([P, H], BF16, tag="bv")
            bc_sb = persist.tile([NC, 1], F32, tag="bc")
            wc_sb = persist.tile([P, HT, NC], BF16, tag="wc")

            def alloc_qk(h):
                qh = qkpool.tile([P, S], BF16, tag="qh", name=f"qh{h}")
                kh = qkpool.tile([P, S], BF16, tag="kh", name=f"kh{h}")
                return qh, kh

            def qk_block(h, wj, b_sb, oT, ss):
                ps = auxps.tile([P, 512], F32, tag="aux",
                                name=f"qk{h}_{ss}")
                for hi in range(HT):
                    nc.tensor.matmul(
                        ps,
                        wj[:, hi, :],
                        xT[:, hi, ss * 512:(ss + 1) * 512],
                        start=(hi == 0),
                        stop=(hi == HT - 1),
                    )
                nc.vector.tensor_tensor(
                    oT[:, ss * 512:(ss + 1) * 512],
                    ps,
                    b_sb[:, h:h + 1].to_broadcast((P, 512)),
                    ALU.add,
                )

            def produce_qk(h, tiles):
                """Project q,k for head h; yields per (w, ss) 8-MM block."""
                qh, kh = tiles
                for w_r, b_sb, oT in ((wq_r, bq_sb, qh), (wk_r, bk_sb, kh)):
                    wj = wjpool.tile([P, HT, P], BF16, tag="wj",
                                     name=f"wj{h}")
                    nc.gpsimd.dma_start(wj, w_r[:, :, h * P:(h + 1) * P])
                    for ss in range(4):
                        qk_block(h, wj, b_sb, oT, ss)
                        yield

            def produce_v_half(dh, sts):
                for st in sts:
                    ps = auxps.tile([P, 512], F32, tag="aux",
                                    name=f"v{dh}_{st}")
                    for hi in range(HT):
                        nc.tensor.matmul(
                            ps,
                            xT[:, hi, st * P:(st + 1) * P],
                            wv_sb[:, hi, dh * 512:(dh + 1) * 512],
                            start=(hi == 0),
                            stop=(hi == HT - 1),
                        )
                    nc.vector.tensor_tensor(
                        v_sb[:, st, dh * 512:(dh + 1) * 512],
                        ps,
                        bv_bc[:, dh * 512:(dh + 1) * 512],
                        ALU.add,
                    )
                    yield

            with tc.tile_pool(name="xload", bufs=1) as xload:
                # gpsimd (casting SWDGE) queue order: q0/k0 weights first so
                # the head-0 projection can start early; V's first half next.
                qk_next = alloc_qk(0)
                bq_f = xload.tile([HT, P], F32, tag="bqf")
                bk_f = xload.tile([HT, P], F32, tag="bkf")
                wjq0 = wjpool.tile([P, HT, P], BF16, tag="wj", name="wj0q")
                nc.gpsimd.dma_start(wjq0, wq_r[:, :, 0:P])
                wjk0 = wjpool.tile([P, HT, P], BF16, tag="wj", name="wj0k")
                nc.gpsimd.dma_start(wjk0, wk_r[:, :, 0:P])
                for hi in range(HT):
                    nc.gpsimd.dma_start(wv_sb[:, hi, 0:512],
                                        wv_r[:, hi, 0:512])
                nc.gpsimd.dma_start(bv_bc, bv[None, :].to_broadcast((P, H)))
                for hi in range(HT):
                    nc.gpsimd.dma_start(wv_sb[:, hi, 512:1024],
                                        wv_r[:, hi, 512:1024])
                nc.gpsimd.dma_start(wc_sb, Wc.rearrange("(j p) c -> p j c", p=P))

                # sync (fast HWDGE) queue: the x stream, with the flat bias
                # rows slotted after x3 (they are tiny but any ring issue
                # delays later x tiles ~2us, so they ride mid-stream; the
                # gpsimd SWDGE ring delivers nothing before ~16us, too late)
                xts = []
                for st in range(ST):
                    xt = xload.tile([P, H], F32, tag=f"xl{st % 8}",
                                    name=f"xt{st}")
                    nc.sync.dma_start(xt, x[st * P:(st + 1) * P, :])
                    xts.append(xt)
                    if st == 3:
                        nc.sync.dma_start(
                            bq_f, bq.rearrange("(j p) -> j p", p=P))
                        nc.sync.dma_start(
                            bk_f, bk.rearrange("(j p) -> j p", p=P))
                        nc.sync.dma_start(bc_sb, bc[:, None])

                def qk0_block256(w, c2):
                    # head-0 q/k in 256-wide blocks: block c2 needs only x
                    # seq tiles 2*c2, 2*c2+1 transposed, so projection work
                    # is available almost immediately as tiles land.
                    wj, b_sb, oT = ((wjq0, bq_sb, qk_next[0]),
                                    (wjk0, bk_sb, qk_next[1]))[w]
                    ps = auxps.tile([P, 256], F32, tag="aux",
                                    name=f"qk0_{w}_{c2}")
                    for hi in range(HT):
                        nc.tensor.matmul(
                            ps,
                            wj[:, hi, :],
                            xT[:, hi, c2 * 256:(c2 + 1) * 256],
                            start=(hi == 0),
                            stop=(hi == HT - 1),
                        )
                    nc.vector.tensor_tensor(
                        oT[:, c2 * 256:(c2 + 1) * 256],
                        ps,
                        b_sb[:, 0:1].to_broadcast((P, 256)),
                        ALU.add,
                    )

                # --- PE-transpose x into [h, s] bf16 layout, interleaved
                # with head-0 q/k projection blocks as seq tiles land ---
                for st in range(ST):
                    xt = xts[st]
                    for jg in range(2):
                        ps = g2ps.tile([P, 4, P], F32, tag="g2")
                        for j4 in range(4):
                            j = jg * 4 + j4
                            nc.tensor.transpose(
                                ps[:, j4, :], xt[:, j * P:(j + 1) * P], ident
                            )
                        # copy on ACT (idle until the first exp ~40us in);
                        # DVE is the chase-phase straggler otherwise
                        nc.scalar.copy(
                            xT[:, jg * 4:(jg + 1) * 4, st * P:(st + 1) * P],
                            ps[:],
                        )
                    if st == 4:
                        # PE-transpose the flat bias rows into [128, 8]
                        for bf, bsb in ((bq_f, bq_sb), (bk_f, bk_sb)):
                            ps8 = g2ps.tile([P, HT], F32, tag="g2",
                                            name="bias8")
                            nc.tensor.transpose(ps8, bf, ident[0:HT, 0:HT])
                            nc.vector.tensor_copy(bsb, ps8)
                    # q/k blocks start once wjq0's slow casting DMA has
                    # landed (~st 7); block c2 only needs x tiles <= 2c2+1,
                    # all transposed+copied well before
                    if st >= 7 and st % 2 == 1:
                        c2 = (st - 7) // 2
                        qk0_block256(0, c2)
                        qk0_block256(1, c2)
                for c2 in (5, 6, 7):
                    qk0_block256(0, c2)
                    qk0_block256(1, c2)

                # --- V first half (heads 0-3) ---
                for _ in produce_v_half(0, range(ST)):
                    pass

            with (
                tc.tile_pool(name="pt", bufs=2) as ptpool,
                tc.tile_pool(name="tree", bufs=1) as treepool,
                tc.tile_pool(name="rc", bufs=2) as rcpool,
            ):
                aux_q = deque()
                aux_q.append(("v1", produce_v_half(1, range(ST))))

                def force_drain(key):
                    for k, g in list(aux_q):
                        if k == key:
                            for _ in g:
                                pass
                            aux_q.remove((k, g))

                def emit_gelu(hh, lo, hi):
                    # gelu (sigmoid approx) of head hh, cols [lo,hi). an
                    # holds attn/2, so s = an'*(1 + tanh(1.7018*an')) ==
                    # gelu(attn).
                    nc.scalar.activation(wg[:, lo:hi], an[:, hh, lo:hi],
                                         AF.Tanh, scale=TANH_SCALE)
                    nc.vector.tensor_tensor(wg[:, lo:hi], an[:, hh, lo:hi],
                                            wg[:, lo:hi], ALU.mult)
                    nc.vector.tensor_tensor(an[:, hh, lo:hi],
                                            an[:, hh, lo:hi], wg[:, lo:hi],
                                            ALU.add)

                def emit_cls(hh, qb2):
                    # classify partial of head hh for one 512-col block,
                    # accumulated into clacc; head 7 streams the finished
                    # block straight out to DRAM.
                    lps = auxps.tile([NC, 512], F32, tag="aux",
                                     name=f"cls{hh}_{qb2}")
                    nc.tensor.matmul(
                        lps,
                        wc_sb[:, hh, :],
                        an[:, hh, qb2 * 512:(qb2 + 1) * 512],
                        start=True, stop=True,
                    )
                    cl = clacc[:, qb2 * 512:(qb2 + 1) * 512]
                    if hh == 0:
                        nc.vector.tensor_tensor(
                            cl, lps, bc_sb.to_broadcast((NC, 512)), ALU.add)
                    else:
                        nc.vector.tensor_tensor(cl, cl, lps, ALU.add)
                    if hh == NH - 1:
                        nc.sync.dma_start(
                            out[:, qb2 * 512:(qb2 + 1) * 512], cl)

                # pipeline state of the previous (head, qb) block
                prev = None  # (h, qb, PT, t1)

                def window(h, qb):
                    nonlocal prev
                    # the very last block uses a finer-grained tree so the
                    # end-of-kernel denominator chain after the last exp is
                    # ~2us shorter
                    fine = (h == NH - 1 and qb == SB - 1)
                    qh, kh = qk_cur
                    PT = ptpool.tile([P, ST, 512], BF16, tag="pt")
                    if prev is not None:
                        ph, pqb, pPT, pt1 = prev
                        pv = pvpool.tile([P, 512], F32, tag="pv")
                        rcb = rcpool.tile([P, 512], F32, tag="rc")
                    tA = tB = tC = tD = None
                    qs = qh[:, qb * 512:(qb + 1) * 512]
                    for kg in range(8):
                        ps = g2ps.tile([P, 2, 512], F32, tag="g2")
                        for k2 in range(2):
                            kt = kg * 2 + k2
                            nc.tensor.matmul(
                                ps[:, k2, :],
                                kh[:, kt * P:(kt + 1) * P],
                                qs,
                                start=True,
                                stop=True,
                            )
                        nc.scalar.activation(
                            PT[:, kg * 2:kg * 2 + 2, :], ps[:], AF.Exp,
                            scale=SCALE,
                        )
                        if prev is not None:
                            # two PV matmuls of the previous block (four in
                            # the fine window, so an-mult/gelu/classify of
                            # the second-to-last block can retire early
                            # instead of serializing after the last exp)
                            n = 4 if fine else 2
                            for k2 in range(n):
                                kt = kg * n + k2
                                if kt >= ST:
                                    break
                                nc.tensor.matmul(
                                    pv,
                                    v_sb[:, kt, ph * DK:(ph + 1) * DK],
                                    pPT[:, kt, :],
                                    start=(kt == 0),
                                    stop=(kt == ST - 1),
                                )
                        if fine and prev is not None and kg == 4:
                            nc.vector.tensor_tensor(
                                an[:, ph, pqb * 512:(pqb + 1) * 512],
                                pv, rcb, ALU.mult)
                        if fine and prev is not None and kg == 5:
                            emit_gelu(ph, pqb * 512, (pqb + 1) * 512)
                        if kg == 3:
                            tA = treepool.tile([P, 4, 512], BF16, tag="tA")
                            nc.vector.tensor_tensor(
                                tA, PT[:, 0:4, :], PT[:, 4:8, :], ALU.add)
                            if prev is not None:
                                denb = auxps.tile([P, 512], F32, tag="aux",
                                                  name=f"dnb{ph}_{pqb}")
                                nc.tensor.matmul(denb, twos128, pt1,
                                                 start=True, stop=True)
                                nc.vector.reciprocal_approx_fast(rcb, denb)
                        elif fine and 4 <= kg <= 6:
                            # kg==7's pair is deferred past an-mult(prev) so
                            # the strict DVE FIFO doesn't make an-mult (and
                            # with it the flush PV) wait on the last exp
                            if kg == 4:
                                tB = treepool.tile([P, 4, 512], BF16,
                                                   tag="tB")
                                tC = treepool.tile([P, 4, 512], BF16,
                                                   tag="tC")
                                tD = treepool.tile([P, 2, 512], BF16,
                                                   tag="tD")
                                nc.vector.tensor_tensor(
                                    tC[:, 0:2, :], tA[:, 0:2, :],
                                    tA[:, 2:4, :], ALU.add)
                            if kg == 5:
                                nc.vector.tensor_tensor(
                                    tC[:, 2, :], tC[:, 0, :], tC[:, 1, :],
                                    ALU.add)
                            nc.vector.tensor_tensor(
                                tB[:, kg - 4, :], PT[:, 2 * kg, :],
                                PT[:, 2 * kg + 1, :], ALU.add)
                            if kg == 6:
                                nc.vector.tensor_tensor(
                                    tD[:, 0, :], tB[:, 0, :], tB[:, 1, :],
                                    ALU.add)
                        # ration aux work (~2 blocks per window) so the
                        # supply lasts through the whole head instead of
                        # front-loading and starving later windows
                        if kg in (1, 5):
                            while aux_q:
                                try:
                                    next(aux_q[0][1])
                                    aux_q.rotate(-1)
                                    break
                                except StopIteration:
                                    aux_q.popleft()
                    # finish previous block: normalize (an' = attn/2 since
                    # rcb = 1/(2*denom))
                    if prev is not None and not fine:
                        nc.vector.tensor_tensor(
                            an[:, ph, pqb * 512:(pqb + 1) * 512],
                            pv, rcb, ALU.mult)
                    # fused denominator tree tail for current block
                    t1 = treepool.tile([P, 512], BF16, tag="tE")
                    if fine:
                        nc.vector.tensor_tensor(
                            tB[:, 3, :], PT[:, 14, :], PT[:, 15, :], ALU.add)
                        nc.vector.tensor_tensor(
                            tD[:, 1, :], tB[:, 2, :], tB[:, 3, :], ALU.add)
                        nc.vector.tensor_tensor(
                            tC[:, 3, :], tD[:, 0, :], tD[:, 1, :], ALU.add)
                        nc.vector.tensor_tensor(
                            t1, tC[:, 2, :], tC[:, 3, :], ALU.add)
                    else:
                        tB = treepool.tile([P, 4, 512], BF16, tag="tB")
                        nc.vector.tensor_tensor(
                            tB, PT[:, 8:12, :], PT[:, 12:16, :], ALU.add)
                        tC = treepool.tile([P, 4, 512], BF16, tag="tC")
                        nc.vector.tensor_tensor(tC, tA, tB, ALU.add)
                        tD = treepool.tile([P, 2, 512], BF16, tag="tD")
                        nc.vector.tensor_tensor(tD, tC[:, 0:2, :],
                                                tC[:, 2:4, :], ALU.add)
                        nc.vector.tensor_tensor(t1, tD[:, 0, :], tD[:, 1, :],
                                                ALU.add)
                    prev = (h, qb, PT, t1)

                qk_cur = qk_next
                gen_qk7b = None
                for h in range(NH):
                    force_drain(f"qk{h}")
                    if h == NH - 1 and gen_qk7b is not None:
                        # queue the deferred head-7 q blocks: they fill the
                        # aux-starved head-7 windows (block ss is pulled two
                        # windows before window (7, ss) consumes it)
                        aux_q.append(("qk7b", gen_qk7b))
                    if h == 4:
                        force_drain("v1")
                    for qb in range(SB):
                        if qb == 1 and h >= 1:
                            emit_gelu(h - 1, 0, S)
                        window(h, qb)
                        if qb == 2 and h >= 1:
                            for qb2 in range(SB):
                                emit_cls(h - 1, qb2)
                        if h == NH - 1 and qb >= 1:
                            # head-7 epilogue per block, staggered so the
                            # tanh/DVE chain hides in the aux-starved last
                            # head's windows instead of serializing the tail
                            # (blocks 2 and 3 are handled by the fine last
                            # window and the flush)
                            if qb <= 2:
                                emit_gelu(h, (qb - 1) * 512, qb * 512)
                            if qb >= 2:
                                emit_cls(h, qb - 2)
                        if qb == 0 and h + 1 < NH:
                            qk_next = alloc_qk(h + 1)
                            if h + 1 == NH - 1:
                                # head 7: produce k (+ q block 0) during head
                                # 6 as usual, but defer q blocks 1-3 into
                                # head 7's own (aux-starved) windows
                                t7 = qk_next
                                wjq7 = wjpool.tile([P, HT, P], BF16,
                                                   tag="wj", name="wj7q")
                                nc.gpsimd.dma_start(
                                    wjq7, wq_r[:, :, (NH - 1) * P:NH * P])
                                wjk7 = wjpool.tile([P, HT, P], BF16,
                                                   tag="wj", name="wj7k")
                                nc.gpsimd.dma_start(
                                    wjk7, wk_r[:, :, (NH - 1) * P:NH * P])

                                def gen_a():
                                    for ss in range(4):
                                        qk_block(NH - 1, wjk7, bk_sb,
                                                 t7[1], ss)
                                        yield
                                    qk_block(NH - 1, wjq7, bq_sb, t7[0], 0)
                                    yield

                                def gen_b():
                                    for ss in (1, 2, 3):
                                        qk_block(NH - 1, wjq7, bq_sb,
                                                 t7[0], ss)
                                        yield

                                aux_q.append((f"qk{h + 1}", gen_a()))
                                gen_qk7b = gen_b()
                            else:
                                aux_q.append(
                                    (f"qk{h + 1}",
                                     produce_qk(h + 1, qk_next))
                                )
                        if h == NH - 1 and qb == 2:
                            force_drain("qk7b")
                    qk_cur = qk_next

                # --- flush: PV + normalize of the last block, then the
                # remaining head-7 epilogue blocks ---
                ph, pqb, pPT, pt1 = prev
                pv = pvpool.tile([P, 512], F32, tag="pv")
                rcb = rcpool.tile([P, 512], F32, tag="rc")
                denb = auxps.tile([P, 512], F32, tag="aux", name="dnb_last")
                # PV first: its early chunks only need early exp groups, so
                # it overlaps the denominator tree/reciprocal of the last
                # block instead of idling behind them
                for kt in range(ST):
                    nc.tensor.matmul(
                        pv,
                        v_sb[:, kt, ph * DK:(ph + 1) * DK],
                        pPT[:, kt, :],
                        start=(kt == 0),
                        stop=(kt == ST - 1),
                    )
                    if kt == 7:
                        emit_cls(NH - 1, 2)
                    if kt == 11:
                        nc.tensor.matmul(denb, twos128, pt1,
                                         start=True, stop=True)
                        nc.vector.reciprocal_approx_fast(rcb, denb)
                # final block: normalize + gelu in halves so the ACT/DVE
                # chain pipelines instead of serializing 512-wide ops
                for hf in range(2):
                    sl0 = pqb * 512 + hf * 256
                    nc.vector.tensor_tensor(
                        an[:, ph, sl0:sl0 + 256], pv[:, hf * 256:(hf + 1) * 256],
                        rcb[:, hf * 256:(hf + 1) * 256], ALU.mult)
                    emit_gelu(NH - 1, sl0, sl0 + 256)
                emit_cls(NH - 1, 3)

    nc.finalize()
    return nc


def get_nc():
    if not _NC_CACHE:
        _NC_CACHE.append(_build())
    return _NC_CACHE[0]


def kernel(**inputs) -> np.ndarray:
    ins = {k: np.ascontiguousarray(np.asarray(v, dtype=np.float32))
           for k, v in inputs.items()}
    x = ins["x"]
    assert x.shape == (B, S, H), x.shape
    shared = {k: ins[k] for k in
              ("Wq", "bq", "Wk", "bk", "Wv", "bv", "Wc", "bc")}
    in_maps = [{"x": x[b], **shared} for b in range(B)]
    nc = get_nc()
    res = run_bass_kernel_spmd(nc, in_maps, core_ids=list(range(B)))
    outs = [np.asarray(res.results[b]["out"], dtype=np.float32).T
            for b in range(B)]
    return np.stack(outs, axis=0)


# revision 50
# speedup vs baseline: 1.0169x; 1.0131x over previous
"""TRN2 Bass kernel for nn_CustomHeadMultiHeadAttention (dense transformer).

Full inputs: x [8, 2048, 1024] f32 + QKV/classify weights. Sharding: pure
data parallelism — batch 8 across 8 NeuronCores, one batch element per core.
Each core runs the complete MHA + GELU + classify on its slice; no
collectives. Host only slices the batch and stacks/transposes the outputs.

Per-core pipeline (bf16 matmul operands, fp32 PSUM accumulation), v2:
  startup: PE-transposes of x seq-tiles, q0/k0 projection blocks and the
    first half of V are interleaved so the PE chases the x DMA stream
    instead of FIFO-stalling on the last tile.
  per head h, per 512-wide q block (software-pipelined one block deep):
    scores^T tiles = kh-chunk (lhsT) @ qh      [k=128, q=512] x2 per group
    P^T  = exp(scores^T / sqrt(dk))            ACT PSUM->SBUF bf16
    PV of the PREVIOUS block is interleaved between score groups, so the
    PE never head-of-line blocks on the exp stream.
    denom = 5-op fused bf16 tree over P^T + twos[128,128] @ t1 broadcast
    (the 2x folds the 0.5 of gelu into the softmax reciprocal)
    an'  = attn^T/2 = pv * reciprocal_approx_fast(2*denom)
  per head epilogue (no ACT table switches: exp/tanh/copy share one set):
    gelu_sigmoid: t = tanh(1.7018*an'); s = an' + an'*t  == gelu(attn)
    classify partial: clacc[2, S] += Wc-chunk(h) lhsT @ s   (DVE accum)
  out = clacc + bc (accumulated from head 0), single DMA [2, S].
Host transposes logits^T [2, s] -> [2048, 2].
"""

import math
import sys
from collections import deque

sys.path.insert(0, "/opt/trn_rl_repo")

import numpy as np

import concourse.bass as bass
import concourse.mybir as mybir
import concourse.tile as tile
from concourse import bacc
from concourse.bass_utils import run_bass_kernel_spmd
from concourse.masks import make_identity

AF = mybir.ActivationFunctionType
ALU = mybir.AluOpType
F32 = mybir.dt.float32
BF16 = mybir.dt.bfloat16

B = 8           # batch (== number of cores)
S = 2048        # sequence length
H = 1024        # hidden
NH = 8          # heads
DK = 128        # head dim
P = 128         # partitions
NC = 2          # classes
SB = S // 512   # 4 q/s blocks of 512
HT = H // P     # 8 hidden tiles
ST = S // P     # 16 seq tiles
SCALE = 1.0 / math.sqrt(DK)
TANH_SCALE = 1.7018  # tanh arg on an' = attn/2: tanh(0.8509 * attn)

_NC_CACHE = []


def _build():
    nc = bacc.Bacc(None, target_bir_lowering=False, debug=False)

    x = nc.dram_tensor("x", [S, H], F32, kind="ExternalInput")
    Wq = nc.dram_tensor("Wq", [H, H], F32, kind="ExternalInput")
    bq = nc.dram_tensor("bq", [H], F32, kind="ExternalInput")
    Wk = nc.dram_tensor("Wk", [H, H], F32, kind="ExternalInput")
    bk = nc.dram_tensor("bk", [H], F32, kind="ExternalInput")
    Wv = nc.dram_tensor("Wv", [H, H], F32, kind="ExternalInput")
    bv = nc.dram_tensor("bv", [H], F32, kind="ExternalInput")
    Wc = nc.dram_tensor("Wc", [H, NC], F32, kind="ExternalInput")
    bc = nc.dram_tensor("bc", [NC], F32, kind="ExternalInput")
    out = nc.dram_tensor("out", [NC, S], F32, kind="ExternalOutput")

    with tile.TileContext(nc) as tc:
        with (
            tc.tile_pool(name="persist", bufs=1) as persist,
            tc.tile_pool(name="g2ps", bufs=2, space="PSUM") as g2ps,
            tc.tile_pool(name="pvps", bufs=2, space="PSUM") as pvpool,
            tc.tile_pool(name="auxps", bufs=2, space="PSUM") as auxps,
            tc.tile_pool(name="wj", bufs=3) as wjpool,
            tc.tile_pool(name="qk", bufs=2) as qkpool,
        ):
            ident = persist.tile([P, P], F32, tag="ident")
            make_identity(nc, ident)
            twos128 = persist.tile([P, P], BF16, tag="twos128")
            nc.vector.memset(twos128, 2.0)

            xT = persist.tile([P, HT, S], BF16, tag="xT")
            wv_sb = persist.tile([P, HT, H], BF16, tag="wv")
            v_sb = persist.tile([P, ST, H], BF16, tag="v")
            an = persist.tile([P, NH, S], BF16, tag="an")
            wg = persist.tile([P, S], BF16, tag="wg")
            clacc = persist.tile([NC, S], F32, tag="clacc")
            wq_r = Wq.rearrange("(o p) d -> p o d", p=P)
            wk_r = Wk.rearrange("(o p) d -> p o d", p=P)
            wv_r = Wv.rearrange("(o p) d -> p o d", p=P)

            bq_sb = persist.tile([P, HT], F32, tag="bq")
            bk_sb = persist.tile([P, HT], F32, tag="bk")
            bv_bc = persist.tile([P, H], BF16, tag="bv")
            bc_sb = persist.tile([NC, 1], F32, tag="bc")
            wc_sb = persist.tile([P, HT, NC], BF16, tag="wc")

            def alloc_qk(h):
                qh = qkpool.tile([P, S], BF16, tag="qh", name=f"qh{h}")
                kh = qkpool.tile([P, S], BF16, tag="kh", name=f"kh{h}")
                return qh, kh

            def qk_block(h, wj, b_sb, oT, ss):
                ps = auxps.tile([P, 512], F32, tag="aux",
                                name=f"qk{h}_{ss}")
                for hi in range(HT):
                    nc.tensor.matmul(
                        ps,
                        wj[:, hi, :],
                        xT[:, hi, ss * 512:(ss + 1) * 512],
                        start=(hi == 0),
                        stop=(hi == HT - 1),
                    )
                nc.vector.tensor_tensor(
                    oT[:, ss * 512:(ss + 1) * 512],
                    ps,
                    b_sb[:, h:h + 1].to_broadcast((P, 512)),
                    ALU.add,
                )

            def produce_qk(h, tiles):
                """Project q,k for head h; yields per (w, ss) 8-MM block."""
                qh, kh = tiles
                for w_r, b_sb, oT in ((wq_r, bq_sb, qh), (wk_r, bk_sb, kh)):
                    wj = wjpool.tile([P, HT, P], BF16, tag="wj",
                                     name=f"wj{h}")
                    nc.gpsimd.dma_start(wj, w_r[:, :, h * P:(h + 1) * P])
                    for ss in range(4):
                        qk_block(h, wj, b_sb, oT, ss)
                        yield

            def produce_v_half(dh, sts):
                for st in sts:
                    ps = auxps.tile([P, 512], F32, tag="aux",
                                    name=f"v{dh}_{st}")
                    for hi in range(HT):
                        nc.tensor.matmul(
                            ps,
                            xT[:, hi, st * P:(st + 1) * P],
                            wv_sb[:, hi, dh * 512:(dh + 1) * 512],
                            start=(hi == 0),
                            stop=(hi == HT - 1),
                        )
                    nc.vector.tensor_tensor(
                        v_sb[:, st, dh * 512:(dh + 1) * 512],
                        ps,
                        bv_bc[:, dh * 512:(dh + 1) * 512],
                        ALU.add,
                    )
                    yield

            with tc.tile_pool(name="xload", bufs=1) as xload:
                # gpsimd (casting SWDGE) queue order: q0/k0 weights first so
                # the head-0 projection can start early; V's first half next.
                qk_next = alloc_qk(0)
                bq_f = xload.tile([HT, P], F32, tag="bqf")
                bk_f = xload.tile([HT, P], F32, tag="bkf")
                wjq0 = wjpool.tile([P, HT, P], BF16, tag="wj", name="wj0q")
                nc.gpsimd.dma_start(wjq0, wq_r[:, :, 0:P])
                wjk0 = wjpool.tile([P, HT, P], BF16, tag="wj", name="wj0k")
                nc.gpsimd.dma_start(wjk0, wk_r[:, :, 0:P])
                for hi in range(HT):
                    nc.gpsimd.dma_start(wv_sb[:, hi, 0:512],
                                        wv_r[:, hi, 0:512])
                nc.gpsimd.dma_start(bv_bc, bv[None, :].to_broadcast((P, H)))
                for hi in range(HT):
                    nc.gpsimd.dma_start(wv_sb[:, hi, 512:1024],
                                        wv_r[:, hi, 512:1024])
                nc.gpsimd.dma_start(wc_sb, Wc.rearrange("(j p) c -> p j c", p=P))

                # sync (fast HWDGE) queue: the x stream, with the flat bias
                # rows slotted after x3 (they are tiny but any ring issue
                # delays later x tiles ~2us, so they ride mid-stream; the
                # gpsimd SWDGE ring delivers nothing before ~16us, too late)
                xts = []
                for st in range(ST):
                    xt = xload.tile([P, H], F32, tag=f"xl{st % 8}",
                                    name=f"xt{st}")
                    nc.sync.dma_start(xt, x[st * P:(st + 1) * P, :])
                    xts.append(xt)
                    if st == 1:
                        nc.sync.dma_start(
                            bq_f, bq.rearrange("(j p) -> j p", p=P))
                        nc.sync.dma_start(
                            bk_f, bk.rearrange("(j p) -> j p", p=P))
                        nc.sync.dma_start(bc_sb, bc[:, None])

                def qk0_block256(w, c2):
                    # head-0 q/k in 256-wide blocks: block c2 needs only x
                    # seq tiles 2*c2, 2*c2+1 transposed, so projection work
                    # is available almost immediately as tiles land.
                    wj, b_sb, oT = ((wjq0, bq_sb, qk_next[0]),
                                    (wjk0, bk_sb, qk_next[1]))[w]
                    ps = auxps.tile([P, 256], F32, tag="aux",
                                    name=f"qk0_{w}_{c2}")
                    for hi in range(HT):
                        nc.tensor.matmul(
                            ps,
                            wj[:, hi, :],
                            xT[:, hi, c2 * 256:(c2 + 1) * 256],
                            start=(hi == 0),
                            stop=(hi == HT - 1),
                        )
                    nc.vector.tensor_tensor(
                        oT[:, c2 * 256:(c2 + 1) * 256],
                        ps,
                        b_sb[:, 0:1].to_broadcast((P, 256)),
                        ALU.add,
                    )

                # --- PE-transpose x into [h, s] bf16 layout, interleaved
                # with head-0 q/k projection blocks as seq tiles land ---
                for st in range(ST):
                    xt = xts[st]
                    for jg in range(2):
                        ps = g2ps.tile([P, 4, P], F32, tag="g2")
                        for j4 in range(4):
                            j = jg * 4 + j4
                            nc.tensor.transpose(
                                ps[:, j4, :], xt[:, j * P:(j + 1) * P], ident
                            )
                        # copy on ACT (idle until the first exp ~40us in);
                        # DVE is the chase-phase straggler otherwise
                        nc.scalar.copy(
                            xT[:, jg * 4:(jg + 1) * 4, st * P:(st + 1) * P],
                            ps[:],
                        )
                    if st == 2:
                        # PE-transpose the flat bias rows into [128, 8]
                        for bf, bsb in ((bq_f, bq_sb), (bk_f, bk_sb)):
                            ps8 = g2ps.tile([P, HT], F32, tag="g2",
                                            name="bias8")
                            nc.tensor.transpose(ps8, bf, ident[0:HT, 0:HT])
                            nc.vector.tensor_copy(bsb, ps8)
                    # q/k block c2 covers x tiles 2c2..2c2+1; running one
                    # tile behind the transposes hides the psum->sbuf copy
                    # latency in the dependency chain
                    if st >= 3 and st % 2 == 1:
                        qk0_block256(0, (st - 3) // 2)
                        qk0_block256(1, (st - 3) // 2)
                for c2 in (7,):
                    qk0_block256(0, c2)
                    qk0_block256(1, c2)

                # --- V first half (heads 0-3) ---
                for _ in produce_v_half(0, range(ST)):
                    pass

            with (
                tc.tile_pool(name="pt", bufs=2) as ptpool,
                tc.tile_pool(name="tree", bufs=1) as treepool,
                tc.tile_pool(name="rc", bufs=2) as rcpool,
            ):
                aux_q = deque()
                aux_q.append(("v1", produce_v_half(1, range(ST))))

                def force_drain(key):
                    for k, g in list(aux_q):
                        if k == key:
                            for _ in g:
                                pass
                            aux_q.remove((k, g))

                def emit_gelu(hh, lo, hi):
                    # gelu (sigmoid approx) of head hh, cols [lo,hi). an
                    # holds attn/2, so s = an'*(1 + tanh(1.7018*an')) ==
                    # gelu(attn).
                    nc.scalar.activation(wg[:, lo:hi], an[:, hh, lo:hi],
                                         AF.Tanh, scale=TANH_SCALE)
                    nc.vector.tensor_tensor(wg[:, lo:hi], an[:, hh, lo:hi],
                                            wg[:, lo:hi], ALU.mult)
                    nc.vector.tensor_tensor(an[:, hh, lo:hi],
                                            an[:, hh, lo:hi], wg[:, lo:hi],
                                            ALU.add)

                def emit_cls(hh, qb2):
                    # classify partial of head hh for one 512-col block,
                    # accumulated into clacc; head 7 streams the finished
                    # block straight out to DRAM.
                    lps = auxps.tile([NC, 512], F32, tag="aux",
                                     name=f"cls{hh}_{qb2}")
                    nc.tensor.matmul(
                        lps,
                        wc_sb[:, hh, :],
                        an[:, hh, qb2 * 512:(qb2 + 1) * 512],
                        start=True, stop=True,
                    )
                    cl = clacc[:, qb2 * 512:(qb2 + 1) * 512]
                    if hh == 0:
                        nc.vector.tensor_tensor(
                            cl, lps, bc_sb.to_broadcast((NC, 512)), ALU.add)
                    else:
                        nc.vector.tensor_tensor(cl, cl, lps, ALU.add)
                    if hh == NH - 1:
                        nc.sync.dma_start(
                            out[:, qb2 * 512:(qb2 + 1) * 512], cl)

                # pipeline state of the previous (head, qb) block
                prev = None  # (h, qb, PT, t1)

                def window(h, qb):
                    nonlocal prev
                    # the very last block uses a finer-grained tree so the
                    # end-of-kernel denominator chain after the last exp is
                    # ~2us shorter
                    fine = (h == NH - 1 and qb == SB - 1)
                    qh, kh = qk_cur
                    PT = ptpool.tile([P, ST, 512], BF16, tag="pt")
                    if prev is not None:
                        ph, pqb, pPT, pt1 = prev
                        pv = pvpool.tile([P, 512], F32, tag="pv")
                        rcb = rcpool.tile([P, 512], F32, tag="rc")
                    tA = tB = tC = tD = None
                    qs = qh[:, qb * 512:(qb + 1) * 512]
                    for kg in range(8):
                        ps = g2ps.tile([P, 2, 512], F32, tag="g2")
                        for k2 in range(2):
                            kt = kg * 2 + k2
                            nc.tensor.matmul(
                                ps[:, k2, :],
                                kh[:, kt * P:(kt + 1) * P],
                                qs,
                                start=True,
                                stop=True,
                            )
                        nc.scalar.activation(
                            PT[:, kg * 2:kg * 2 + 2, :], ps[:], AF.Exp,
                            scale=SCALE,
                        )
                        if prev is not None:
                            # two PV matmuls of the previous block (four in
                            # the fine window, so an-mult/gelu/classify of
                            # the second-to-last block can retire early
                            # instead of serializing after the last exp)
                            n = 4 if fine else 2
                            for k2 in range(n):
                                kt = kg * n + k2
                                if kt >= ST:
                                    break
                                nc.tensor.matmul(
                                    pv,
                                    v_sb[:, kt, ph * DK:(ph + 1) * DK],
                                    pPT[:, kt, :],
                                    start=(kt == 0),
                                    stop=(kt == ST - 1),
                                )
                        if fine and prev is not None and kg == 4:
                            nc.vector.tensor_tensor(
                                an[:, ph, pqb * 512:(pqb + 1) * 512],
                                pv, rcb, ALU.mult)
                        if fine and prev is not None and kg == 5:
                            emit_gelu(ph, pqb * 512, (pqb + 1) * 512)
                        if kg == 3:
                            tA = treepool.tile([P, 4, 512], BF16, tag="tA")
                            nc.vector.tensor_tensor(
                                tA, PT[:, 0:4, :], PT[:, 4:8, :], ALU.add)
                            if prev is not None:
                                denb = auxps.tile([P, 512], F32, tag="aux",
                                                  name=f"dnb{ph}_{pqb}")
                                nc.tensor.matmul(denb, twos128, pt1,
                                                 start=True, stop=True)
                                nc.vector.reciprocal_approx_fast(rcb, denb)
                        elif fine and 4 <= kg <= 6:
                            # kg==7's pair is deferred past an-mult(prev) so
                            # the strict DVE FIFO doesn't make an-mult (and
                            # with it the flush PV) wait on the last exp
                            if kg == 4:
                                tB = treepool.tile([P, 4, 512], BF16,
                                                   tag="tB")
                                tC = treepool.tile([P, 4, 512], BF16,
                                                   tag="tC")
                                tD = treepool.tile([P, 2, 512], BF16,
                                                   tag="tD")
                                nc.vector.tensor_tensor(
                                    tC[:, 0:2, :], tA[:, 0:2, :],
                                    tA[:, 2:4, :], ALU.add)
                            if kg == 5:
                                nc.vector.tensor_tensor(
                                    tC[:, 2, :], tC[:, 0, :], tC[:, 1, :],
                                    ALU.add)
                            nc.vector.tensor_tensor(
                                tB[:, kg - 4, :], PT[:, 2 * kg, :],
                                PT[:, 2 * kg + 1, :], ALU.add)
                            if kg == 6:
                                nc.vector.tensor_tensor(
                                    tD[:, 0, :], tB[:, 0, :], tB[:, 1, :],
                                    ALU.add)
                        # ration aux work (~2 blocks per window) so the
                        # supply lasts through the whole head instead of
                        # front-loading and starving later windows
                        if kg in (1, 5):
                            while aux_q:
                                try:
                                    next(aux_q[0][1])
                                    aux_q.rotate(-1)
                                    break
                                except StopIteration:
                                    aux_q.popleft()
                    # finish previous block: normalize (an' = attn/2 since
                    # rcb = 1/(2*denom))
                    if prev is not None and not fine:
                        nc.vector.tensor_tensor(
                            an[:, ph, pqb * 512:(pqb + 1) * 512],
                            pv, rcb, ALU.mult)
                    # fused denominator tree tail for current block
                    t1 = treepool.tile([P, 512], BF16, tag="tE")
                    if fine:
                        nc.vector.tensor_tensor(
                            tB[:, 3, :], PT[:, 14, :], PT[:, 15, :], ALU.add)
                        nc.vector.tensor_tensor(
                            tD[:, 1, :], tB[:, 2, :], tB[:, 3, :], ALU.add)
                        nc.vector.tensor_tensor(
                            tC[:, 3, :], tD[:, 0, :], tD[:, 1, :], ALU.add)
                        nc.vector.tensor_tensor(
                            t1, tC[:, 2, :], tC[:, 3, :], ALU.add)
                    else:
                        tB = treepool.tile([P, 4, 512], BF16, tag="tB")
                        nc.vector.tensor_tensor(
                            tB, PT[:, 8:12, :], PT[:, 12:16, :], ALU.add)
                        tC = treepool.tile([P, 4, 512], BF16, tag="tC")
                        nc.vector.tensor_tensor(tC, tA, tB, ALU.add)
                        tD = treepool.tile([P, 2, 512], BF16, tag="tD")
                        nc.vector.tensor_tensor(tD, tC[:, 0:2, :],
                                                tC[:, 2:4, :], ALU.add)
                        nc.vector.tensor_tensor(t1, tD[:, 0, :], tD[:, 1, :],
                                                ALU.add)
                    prev = (h, qb, PT, t1)

                qk_cur = qk_next
                gen_qk7b = None
                for h in range(NH):
                    force_drain(f"qk{h}")
                    if h == NH - 1 and gen_qk7b is not None:
                        # queue the deferred head-7 q blocks: they fill the
                        # aux-starved head-7 windows (block ss is pulled two
                        # windows before window (7, ss) consumes it)
                        aux_q.append(("qk7b", gen_qk7b))
                    if h == 4:
                        force_drain("v1")
                    for qb in range(SB):
                        if qb == 1 and h >= 1:
                            emit_gelu(h - 1, 0, S)
                        window(h, qb)
                        if qb == 2 and h >= 1:
                            for qb2 in range(SB):
                                emit_cls(h - 1, qb2)
                        if h == NH - 1 and qb >= 1:
                            # head-7 epilogue per block, staggered so the
                            # tanh/DVE chain hides in the aux-starved last
                            # head's windows instead of serializing the tail
                            # (blocks 2 and 3 are handled by the fine last
                            # window and the flush)
                            if qb <= 2:
                                emit_gelu(h, (qb - 1) * 512, qb * 512)
                            if qb >= 2:
                                emit_cls(h, qb - 2)
                        if qb == 0 and h + 1 < NH:
                            qk_next = alloc_qk(h + 1)
                            if h + 1 == NH - 1:
                                # head 7: produce k (+ q block 0) during head
                                # 6 as usual, but defer q blocks 1-3 into
                                # head 7's own (aux-starved) windows
                                t7 = qk_next
                                wjq7 = wjpool.tile([P, HT, P], BF16,
                                                   tag="wj", name="wj7q")
                                nc.gpsimd.dma_start(
                                    wjq7, wq_r[:, :, (NH - 1) * P:NH * P])
                                wjk7 = wjpool.tile([P, HT, P], BF16,
                                                   tag="wj", name="wj7k")
                                nc.gpsimd.dma_start(
                                    wjk7, wk_r[:, :, (NH - 1) * P:NH * P])

                                def gen_a():
                                    for ss in range(4):
                                        qk_block(NH - 1, wjk7, bk_sb,
                                                 t7[1], ss)
                                        yield
                                    qk_block(NH - 1, wjq7, bq_sb, t7[0], 0)
                                    yield

                                def gen_b():
                                    for ss in (1, 2, 3):
                                        qk_block(NH - 1, wjq7, bq_sb,
                                                 t7[0], ss)
                                        yield

                                aux_q.append((f"qk{h + 1}", gen_a()))
                                gen_qk7b = gen_b()
                            else:
                                aux_q.append(
                                    (f"qk{h + 1}",
                                     produce_qk(h + 1, qk_next))
                                )
                        if h == NH - 1 and qb == 2:
                            force_drain("qk7b")
                    qk_cur = qk_next

                # --- flush: PV + normalize of the last block, then the
                # remaining head-7 epilogue blocks ---
                ph, pqb, pPT, pt1 = prev
                pv = pvpool.tile([P, 512], F32, tag="pv")
                rcb = rcpool.tile([P, 512], F32, tag="rc")
                denb = auxps.tile([P, 512], F32, tag="aux", name="dnb_last")
                # PV first: its early chunks only need early exp groups, so
                # it overlaps the denominator tree/reciprocal of the last
                # block instead of idling behind them
                for kt in range(ST):
                    nc.tensor.matmul(
                        pv,
                        v_sb[:, kt, ph * DK:(ph + 1) * DK],
                        pPT[:, kt, :],
                        start=(kt == 0),
                        stop=(kt == ST - 1),
                    )
                    if kt == 7:
                        emit_cls(NH - 1, 2)
                    if kt == 11:
                        nc.tensor.matmul(denb, twos128, pt1,
                                         start=True, stop=True)
                        nc.vector.reciprocal_approx_fast(rcb, denb)
                # final block: normalize + gelu in halves so the ACT/DVE
                # chain pipelines instead of serializing 512-wide ops
                for hf in range(2):
                    sl0 = pqb * 512 + hf * 256
                    nc.vector.tensor_tensor(
                        an[:, ph, sl0:sl0 + 256], pv[:, hf * 256:(hf + 1) * 256],
                        rcb[:, hf * 256:(hf + 1) * 256], ALU.mult)
                    emit_gelu(NH - 1, sl0, sl0 + 256)
                emit_cls(NH - 1, 3)

    nc.finalize()
    return nc


def get_nc():
    if not _NC_CACHE:
        _NC_CACHE.append(_build())
    return _NC_CACHE[0]


def kernel(**inputs) -> np.ndarray:
    ins = {k: np.ascontiguousarray(np.asarray(v, dtype=np.float32))
           for k, v in inputs.items()}
    x = ins["x"]
    assert x.shape == (B, S, H), x.shape
    shared = {k: ins[k] for k in
              ("Wq", "bq", "Wk", "bk", "Wv", "bv", "Wc", "bc")}
    in_maps = [{"x": x[b], **shared} for b in range(B)]
    nc = get_nc()
    res = run_bass_kernel_spmd(nc, in_maps, core_ids=list(range(B)))
    outs = [np.asarray(res.results[b]["out"], dtype=np.float32).T
            for b in range(B)]
    return np.stack(outs, axis=0)
